# revision 1
# baseline (speedup 1.0000x reference)
"""AttentiveProtoFusion kernel for 8 TRN2 NeuronCores.

Math (equivalent to reference, ~14x fewer FLOPs):
    q  = sent @ Wq + bq                      [n, 768]
    q' = q @ Wk^T                            [n, 768]
    scores[n,p] = sum_c proto[n,p,c] * q'[n,c]   (+ q.bk, constant over p ->
                                                  dropped: softmax invariant)
    w = softmax(scores, axis=p)
    ctx[n,c] = sum_p w[n,p] * proto[n,p,c]

Sharding: pure data-parallel over the 2048 tokens (B*S), 256 tokens/core.
Per core, tokens live on partitions, in 2 blocks of 128. sent and Wk are
staged host-side in transposed layout (pure relayout; same bytes DMA'd)
so the TensorEngine does no transposes at all.

The softmax-weighted pooling runs ONLINE over chunks of CH prototypes
with a fixed exponent frame Mhat = max(chunk0)+60 (statistically safe:
scores are N(0, ||q'||^2) per token; a later score would need a ~4.8
sigma excursion past the chunk-0 max to overflow, and Z >= e^-60 keeps
well clear of denormals; U/Z equals softmax exactly). Proto tiles are
consumed and their SBUF slots recycled as soon as their chunk is done -
no block-wide barrier.

Engine plan:
  PE    : the two small projection matmuls (no transposes).
  DVE   : affine_mul_reduce (custom fused mul+reduce) for most scores;
          fused MAC (scalar_tensor_tensor) on ctx cols [0:A].
  ACT   : exp; per-partition-scale multiplies for ctx cols [A:]; the
          accum-reduce for GPSIMD-computed score products.
  GPSIMD: a slice of the score multiplies + ctx accumulate adds [A:].
  DMA   : streams proto (24.6 MB/core) - the roofline.
"""

import sys

for _p in ("/opt/trn_rl_repo", "/opt/pypackages"):
    if _p not in sys.path:
        sys.path.append(_p)

import numpy as np

B, S, P, D_SENT, D_CTX = 4, 512, 32, 1024, 768
N_CORES = 8
TOK = B * S                    # 2048
TPC = TOK // N_CORES           # 256 tokens per core
BLK = 128                      # tokens per block
NBLK = TPC // BLK              # 2
PG = 2                         # prototypes per DMA tile
NPG = P // PG                  # 16 proto tiles per block
CH = 8                         # prototypes per online chunk
NCH = P // CH                  # 4 chunks per block
TPCH = CH // PG                # tiles per chunk
PPOOL_BUFS = 18

CTX_DV = 768                   # all ctx adds on DVE (GPSIMD add rate is poor)
GPS_SCORE_P = {15, 19}   # scores routed GPS+ACT (not chunk 0 or 3)

_NC = None


def _build():
    import concourse.bass as bass
    import concourse.tile as tile
    from concourse import bacc, mybir

    f32 = mybir.dt.float32
    Alu = mybir.AluOpType
    Act = mybir.ActivationFunctionType
    X = mybir.AxisListType.X

    nc = bacc.Bacc("TRN2", target_bir_lowering=False)

    sentT_d = nc.dram_tensor("sentT", [D_SENT, TPC], f32, kind="ExternalInput")
    proto_d = nc.dram_tensor("proto", [TPC, P, D_CTX], f32, kind="ExternalInput")
    w_d = nc.dram_tensor("w", [D_SENT, D_CTX], f32, kind="ExternalInput")
    bp_d = nc.dram_tensor("bp", [1, D_CTX], f32, kind="ExternalInput")
    out_d = nc.dram_tensor("out", [TPC, D_CTX], f32, kind="ExternalOutput")

    DS = D_SENT // 128   # 8 chunks of the sent feature dim
    DC = D_CTX // 128    # 6 chunks of the ctx feature dim
    EH = D_CTX // 2      # 384

    with tile.TileContext(nc) as tc:
        with (
            tc.tile_pool(name="persist", bufs=1) as persist,
            tc.tile_pool(name="wpool", bufs=1) as wpool,
            tc.tile_pool(name="ppool", bufs=PPOOL_BUFS) as ppool,
            tc.tile_pool(name="small", bufs=4) as small,
            tc.tile_pool(name="scratch", bufs=2) as scratch,
            tc.tile_pool(name="tmpp", bufs=3) as tmpp,
            tc.tile_pool(name="psum", bufs=2, space="PSUM") as psum,
        ):
            qp_sb = persist.tile([128, NBLK, D_CTX], f32)   # q' per block [n, e]
            scores = persist.tile([128, NBLK, P], f32)
            expw = persist.tile([128, NBLK, P], f32)        # exp(s - Mhat)
            U = persist.tile([128, NBLK, D_CTX], f32)       # ctx numerator
            negMhat = persist.tile([128, NBLK, 1], f32)
            clampv = persist.tile([128, NBLK, 1], f32)      # Mhat + 80
            Zrun = persist.tile([128, NBLK, 1], f32)

            # ------------- weights + projection (folded, no transposes) ---
            # qp[n, e] = sum_d sent[n, d] * W[d, e] + bp[e],
            # W = Wq @ Wk^T and bp = bq @ Wk^T folded host-side.
            sentT_sb = wpool.tile([128, DS, TPC], f32)      # sent^T[(dd p), n]
            nc.sync.dma_start(
                out=sentT_sb[:],
                in_=sentT_d[:].rearrange("(dd p) n -> p dd n", p=128),
            )
            w_sb = wpool.tile([128, DS, D_CTX], f32)        # W[(dd p), e]
            nc.sync.dma_start(
                out=w_sb[:], in_=w_d[:].rearrange("(dd p) e -> p dd e", p=128)
            )
            bp_sb = wpool.tile([1, D_CTX], f32)
            nc.sync.dma_start(out=bp_sb[:], in_=bp_d[:])
            ones_sb = wpool.tile([1, 128], f32)
            nc.vector.memset(ones_sb[:], 1.0)

            for b in range(NBLK):
                for h in range(2):
                    pp = psum.tile([128, EH], f32, tag="mm")
                    for dd in range(DS):
                        nc.tensor.matmul(
                            pp[:],
                            sentT_sb[:, dd, b * BLK:(b + 1) * BLK],
                            w_sb[:, dd, h * EH:(h + 1) * EH],
                            start=(dd == 0),
                            stop=False,
                        )
                    nc.tensor.matmul(
                        pp[:],
                        ones_sb[0:1, :],
                        bp_sb[0:1, h * EH:(h + 1) * EH],
                        start=False,
                        stop=True,
                    )
                    nc.scalar.copy(out=qp_sb[:, b, h * EH:(h + 1) * EH], in_=pp[:])

            # ---------------- main loop: online softmax-pooling ----------
            for c in range(NCH):
                for b in range(NBLK):
                    t_tiles = []
                    for t in range(TPCH):
                        g = c * TPCH + t
                        T4 = ppool.tile([128, PG, D_CTX], f32, tag="T")
                        nc.sync.dma_start(
                            out=T4[:],
                            in_=proto_d[
                                b * BLK:(b + 1) * BLK, g * PG:(g + 1) * PG, :
                            ],
                        )
                        t_tiles.append(T4)
                        for j in range(PG):
                            p = g * PG + j
                            if p in GPS_SCORE_P:
                                gs = tmpp.tile([128, D_CTX], f32, tag="gscore")
                                nc.gpsimd.tensor_tensor(
                                    out=gs[:], in0=T4[:, j, :],
                                    in1=qp_sb[:, b, :], op=Alu.mult,
                                )
                                nc.scalar.activation(
                                    out=gs[:], in_=gs[:], func=Act.Copy,
                                    accum_out=scores[:, b, p:p + 1],
                                )
                            else:
                                amr_out = scratch.tile(
                                    [128, D_CTX], f32, tag="amr_out"
                                )
                                nc.vector.affine_mul_reduce(
                                    out=amr_out[:],
                                    accum_out=scores[:, b, p:p + 1],
                                    in0=T4[:, j, :],
                                    in1=qp_sb[:, b, :],
                                    scale=1.0,
                                    bias=0.0,
                                )

                    s_ch = scores[:, b, c * CH:(c + 1) * CH]
                    e_ch = expw[:, b, c * CH:(c + 1) * CH]
                    if c == 0:
                        # fixed frame Mhat = max(chunk0) + 30 (see header)
                        m8 = small.tile([128, 1], f32, tag="m8")
                        nc.vector.tensor_reduce(
                            out=m8[:], in_=s_ch, axis=X, op=Alu.max,
                        )
                        # negMhat = -(max + 60); clampv = max + 140
                        nc.vector.tensor_scalar(
                            negMhat[:, b, :], m8[:], -1.0, -60.0,
                            Alu.mult, Alu.add,
                        )
                        nc.vector.tensor_scalar(
                            clampv[:, b, :], m8[:], 1.0, 140.0,
                            Alu.mult, Alu.add,
                        )
                        nc.scalar.activation(
                            out=e_ch, in_=s_ch, func=Act.Exp,
                            bias=negMhat[:, b, :], scale=1.0,
                        )
                        nc.vector.tensor_reduce(
                            out=Zrun[:, b, :], in_=e_ch, axis=X, op=Alu.add,
                        )
                    else:
                        # guard the fixed frame: s <= Mhat + 80 so exp can
                        # never overflow even for extreme outliers
                        nc.vector.tensor_scalar(
                            s_ch, s_ch, clampv[:, b, :], None, Alu.min,
                        )
                        nc.scalar.activation(
                            out=e_ch, in_=s_ch, func=Act.Exp,
                            bias=negMhat[:, b, :], scale=1.0,
                        )
                        zloc = small.tile([128, 1], f32, tag="zloc")
                        nc.vector.tensor_reduce(
                            out=zloc[:], in_=e_ch, axis=X, op=Alu.add,
                        )
                        nc.vector.tensor_tensor(
                            out=Zrun[:, b, :], in0=Zrun[:, b, :], in1=zloc[:],
                            op=Alu.add,
                        )

                    # MACs: U += e_p * T_p  (ACT multiplies, DVE+GPS add)
                    DV = CTX_DV
                    for t in range(TPCH):
                        T4 = t_tiles[t]
                        for j in range(PG):
                            p = (c * TPCH + t) * PG + j
                            e_p = expw[:, b, p:p + 1]
                            if p == 0:
                                nc.scalar.activation(
                                    out=U[:, b, :], in_=T4[:, j, :],
                                    func=Act.Copy, scale=e_p,
                                )
                            else:
                                gtmp = tmpp.tile([128, D_CTX], f32, tag="gtmp")
                                nc.scalar.activation(
                                    out=gtmp[:], in_=T4[:, j, :],
                                    func=Act.Copy, scale=e_p,
                                )
                                nc.vector.tensor_tensor(
                                    out=U[:, b, 0:DV], in0=gtmp[:, 0:DV],
                                    in1=U[:, b, 0:DV], op=Alu.add,
                                )
                                if DV < D_CTX:
                                    nc.gpsimd.tensor_tensor(
                                        out=U[:, b, DV:], in0=gtmp[:, DV:],
                                        in1=U[:, b, DV:], op=Alu.add,
                                    )

            # -- finalize: ctx = U / Z --
            for b in range(NBLK):
                rinv = small.tile([128, 1], f32, tag="rinv")
                nc.vector.reciprocal(out=rinv[:], in_=Zrun[:, b, :])
                nc.vector.tensor_scalar(
                    U[:, b, 0:384], U[:, b, 0:384], rinv[:], None, Alu.mult,
                )
                nc.scalar.activation(
                    out=U[:, b, 384:], in_=U[:, b, 384:], func=Act.Copy,
                    scale=rinv[:],
                )
                nc.sync.dma_start(
                    out=out_d[b * BLK:(b + 1) * BLK, :], in_=U[:, b, :]
                )

    nc.compile()
    return nc


def _get_nc():
    global _NC
    if _NC is None:
        _NC = _build()
    return _NC


def _make_in_maps(sent_vecs, proto_vecs, Wq, bq, Wk):
    sent = np.asarray(sent_vecs, dtype=np.float32).reshape(TOK, D_SENT)
    sentT = np.ascontiguousarray(sent.T)                      # [D_SENT, TOK]
    proto = np.ascontiguousarray(
        np.asarray(proto_vecs, dtype=np.float32).reshape(TOK, P, D_CTX)
    )
    wq = np.asarray(Wq, dtype=np.float32)
    bq = np.asarray(bq, dtype=np.float32).reshape(1, D_CTX)
    wk = np.asarray(Wk, dtype=np.float32)
    # fold the projection weights host-side: qp = sent @ W + bp
    w = np.ascontiguousarray(wq @ wk.T)
    bp = np.ascontiguousarray(bq @ wk.T)
    in_maps = []
    for i in range(N_CORES):
        sl = slice(i * TPC, (i + 1) * TPC)
        in_maps.append(
            {
                "sentT": np.ascontiguousarray(sentT[:, sl]),
                "proto": np.ascontiguousarray(proto[sl]),
                "w": w,
                "bp": bp,
            }
        )
    return in_maps


def _ensure_ntff_hook():
    """The agent image's antenv lacks axon_hooks; shim it so trace=True
    can capture NTFF profiles via the libaxon ctypes path."""
    try:
        from antenv.axon_hooks import get_axon_ntff_profile_hook  # noqa: F401
        return
    except ImportError:
        pass
    import types

    import antenv
    from trn_agent_boot.trn_boot import _ntff_profile_via_ctypes

    mod = types.ModuleType("antenv.axon_hooks")
    mod._hook = _ntff_profile_via_ctypes("/opt/axon/libaxon_pjrt.so")
    mod.get_axon_ntff_profile_hook = lambda: mod._hook
    mod.set_axon_ntff_profile_hook = lambda h: setattr(mod, "_hook", h)
    sys.modules["antenv.axon_hooks"] = mod
    antenv.axon_hooks = mod


def run(sent_vecs, proto_vecs, Wq, bq, Wk, bk=None, trace=False, **kw):
    """Returns (out[4,512,768] float32, BassKernelResults)."""
    from concourse.bass_utils import run_bass_kernel_spmd

    if trace:
        _ensure_ntff_hook()
    nc = _get_nc()
    in_maps = _make_in_maps(sent_vecs, proto_vecs, Wq, bq, Wk)
    res = run_bass_kernel_spmd(
        nc, in_maps, core_ids=list(range(N_CORES)), trace=trace
    )
    outs = [np.asarray(res.results[i]["out"]) for i in range(N_CORES)]
    full = np.concatenate(outs, axis=0).reshape(B, S, D_CTX).astype(np.float32)
    return full, res


def kernel(sent_vecs, proto_vecs, Wq, bq, Wk, bk=None, **kw):
    out, _ = run(sent_vecs, proto_vecs, Wq, bq, Wk, bk)
    return out


if __name__ == "__main__":
    nc = _get_nc()
    print("build + compile OK")



# revision 7
# speedup vs baseline: 1.4928x; 1.4928x over previous
"""AttentiveProtoFusion kernel for 8 TRN2 NeuronCores (v2).

Math (identical algebra to v1, ~14x fewer FLOPs than the reference):
    qp = sent @ (Wq @ Wk^T) + bq @ Wk^T              [n, 768]
    scores[n,p] = sum_c proto[n,p,c] * qp[n,c]   (+ qp.bk const -> dropped)
    w = softmax(scores, axis=p)
    ctx[n,c] = sum_p w[n,p] * proto[n,p,c]

v2 changes vs the 164us v1 baseline (which was DVE-bound at 141us busy):
  * proto / sent / W are staged host-side in fp16 (pure dtype relayout of
    the same values; end-to-end numpy sim of the full fp16 pipeline gives
    rel err 2.0e-3 vs the 2e-2 gate). Halves the DMA roofline to ~15.5 MB
    /core and doubles DVE tensor_tensor throughput (2x_1p mode).
  * The softmax-weighted pooling moves OFF the vector engines entirely:
    U[n,c] += e[n,p] * T_p[n,c] is a TensorE matmul with a DIAGONAL
    stationary matrix diag(e[:,p]) (built by one 4x-mode tensor_scalar
    from a staged identity tile), PSUM-accumulated over all 32 protos.
    out[i,j] = sum_k diag[k,i] * T[k,j] = e[i] * T[i,j].  ~160ns/matmul
    on the otherwise-idle PE instead of ~1us/proto on DVE.
  * Scores stay elementwise (batched per-token dots cannot be a matmul):
    fp16 TT multiply on DVE (2x) with the 768-wide sum routed per-proto
    to ACT (activation Copy accum_out), GPSIMD (tensor_reduce), or fused
    into a 1x DVE affine_mul_reduce — balanced so no engine exceeds the
    ~44us DMA roofline.
  * Online softmax with a fixed exponent frame (Mhat = max(chunk0)+60,
    scores clamped at Mhat+80) as in v1: exp(s-Mhat) <= e^80 fits f32 and
    bf16 (diag dtype); U/Z equals softmax exactly.

Sharding: pure data-parallel over the 2048 tokens (B*S), 256 tokens/core,
2 blocks of 128 tokens on partitions.
"""

import sys

for _p in ("/opt/trn_rl_repo", "/opt/pypackages"):
    if _p not in sys.path:
        sys.path.append(_p)

import numpy as np

B, S, P, D_SENT, D_CTX = 4, 512, 32, 1024, 768
N_CORES = 8
TOK = B * S                    # 2048
TPC = TOK // N_CORES           # 256 tokens per core
BLK = 128                      # tokens per block
NBLK = TPC // BLK              # 2
PG = 4                         # prototypes per DMA tile
NPG = P // PG                  # 8 proto tiles per block
CH = 8                         # prototypes per online chunk
NCH = P // CH                  # 4 chunks per block
TPCH = CH // PG                # 2 tiles per chunk
EH = D_CTX // 2                # 384 (psum-bank-sized half of d_ctx)
DS = D_SENT // 128             # 8 contraction chunks for the projection

# Per-(block,proto) score routing (Q7 ucode only implements tensor_tensor,
# so GPSIMD can only take the 768-wide product, never the reduction):
#   'amr' : fused mult+reduce on DVE (1x, ~955ns, no ACT/GPS)
#   'va'  : DVE fp16 mult (2x, ~555ns) + ACT reduce (Copy accum_out ~1.05us)
#   'ga'  : GPS mult (~1.7us) + ACT reduce
# Balanced so DVE/ACT/GPS all stay under the ~44us DMA roofline.
_CHUNK_PATTERN = ["amr", "ga", "amr", "ga", "amr", "ga", "va", "amr"]
SCORE_ROUTE = {}
for _p_ in range(P):
    for _b_ in range(NBLK):
        SCORE_ROUTE[(_b_, _p_)] = _CHUNK_PATTERN[_p_ % CH]
# diag(e_p) builder engine: 'v' DVE tensor_scalar (4x, ~94ns),
# 'a' ACT activation-copy-scale (~400ns).
DIAG_ROUTE = {}
for _p_ in range(P):
    for _b_ in range(NBLK):
        DIAG_ROUTE[(_b_, _p_)] = "v"

_NC = None


def _build():
    import concourse.bass as bass
    import concourse.tile as tile
    from concourse import bacc, mybir

    f32 = mybir.dt.float32
    f16 = mybir.dt.float16
    bf16 = mybir.dt.bfloat16
    Alu = mybir.AluOpType
    Act = mybir.ActivationFunctionType
    X = mybir.AxisListType.X

    nc = bacc.Bacc("TRN2", target_bir_lowering=False)

    sentT_d = nc.dram_tensor("sentT", [D_SENT, TPC], f16, kind="ExternalInput")
    proto_d = nc.dram_tensor("proto", [TPC, P, D_CTX], f16, kind="ExternalInput")
    w_d = nc.dram_tensor("w", [D_SENT, D_CTX], f16, kind="ExternalInput")
    bp_d = nc.dram_tensor("bp", [1, D_CTX], f16, kind="ExternalInput")
    eye_d = nc.dram_tensor("eye", [BLK, BLK], bf16, kind="ExternalInput")
    out_d = nc.dram_tensor("out", [TPC, D_CTX], f32, kind="ExternalOutput")

    with tile.TileContext(nc) as tc:
        with (
            tc.tile_pool(name="wpool", bufs=1) as wpool,
            tc.tile_pool(name="persist", bufs=1) as persist,
            tc.tile_pool(name="ppool", bufs=NBLK * NPG) as ppool,
            tc.tile_pool(name="prodp", bufs=6) as prodp,
            tc.tile_pool(name="diagp", bufs=6) as diagp,
            tc.tile_pool(name="small", bufs=4) as small,
            tc.tile_pool(name="psq", bufs=2, space="PSUM") as psq,
            tc.tile_pool(name="psu", bufs=1, space="PSUM") as psu,
        ):
            # ---- weights / staged constants --------------------------------
            sentT_sb = wpool.tile([128, DS, TPC], f16)
            nc.sync.dma_start(
                out=sentT_sb[:],
                in_=sentT_d[:].rearrange("(dd p) n -> p dd n", p=128),
            )
            w_sb = wpool.tile([128, DS, D_CTX], f16)
            nc.sync.dma_start(
                out=w_sb[:], in_=w_d[:].rearrange("(dd p) e -> p dd e", p=128)
            )
            bp_sb = wpool.tile([1, D_CTX], f16)
            nc.sync.dma_start(out=bp_sb[:], in_=bp_d[:])
            eye_sb = wpool.tile([128, BLK], bf16)
            nc.sync.dma_start(out=eye_sb[:], in_=eye_d[:])
            ones_sb = wpool.tile([1, 128], f16)
            nc.vector.memset(ones_sb[:], 1.0)

            qp_sb = persist.tile([128, NBLK, D_CTX], f16)
            scores = persist.tile([128, NBLK, P], f32)
            expw = persist.tile([128, NBLK, P], f32)
            negMhat = persist.tile([128, NBLK, 1], f32)
            clampv = persist.tile([128, NBLK, 1], f32)
            Zrun = persist.tile([128, NBLK, 1], f32)
            outsb = persist.tile([128, NBLK, D_CTX], f32)

            # ---- projection: qp = sent @ W + bp  (PE, fp16) ----------------
            for b in range(NBLK):
                for h in range(2):
                    pp = psq.tile([128, EH], f32, tag="mm")
                    for dd in range(DS):
                        nc.tensor.matmul(
                            pp[:],
                            sentT_sb[:, dd, b * BLK:(b + 1) * BLK],
                            w_sb[:, dd, h * EH:(h + 1) * EH],
                            start=(dd == 0),
                            stop=False,
                        )
                    nc.tensor.matmul(
                        pp[:],
                        ones_sb[0:1, :],
                        bp_sb[0:1, h * EH:(h + 1) * EH],
                        start=False,
                        stop=True,
                    )
                    nc.scalar.activation(
                        out=qp_sb[:, b, h * EH:(h + 1) * EH], in_=pp[:],
                        func=Act.Copy,
                    )

            # persistent PSUM accumulators for the pooled context
            U = []
            for b in range(NBLK):
                row = []
                for h in range(2):
                    ubh = psu.tile([128, EH], f32, tag=f"U{b}{h}", name=f"U{b}{h}")
                    row.append(ubh)
                U.append(row)

            # ---- main loop: stream protos; scores -> exp -> PE pooling -----
            for b in range(NBLK):
                for c in range(NCH):
                    t_tiles = []
                    for t in range(TPCH):
                        g = c * TPCH + t
                        T4 = ppool.tile([128, PG, D_CTX], f16, tag="T")
                        nc.sync.dma_start(
                            out=T4[:],
                            in_=proto_d[
                                b * BLK:(b + 1) * BLK, g * PG:(g + 1) * PG, :
                            ],
                        )
                        t_tiles.append(T4)
                        for j in range(PG):
                            p = g * PG + j
                            route = SCORE_ROUTE[(b, p)]
                            if route == "amr":
                                prod = prodp.tile([128, D_CTX], f16, tag="pr")
                                nc.vector.affine_mul_reduce(
                                    out=prod[:],
                                    accum_out=scores[:, b, p:p + 1],
                                    in0=T4[:, j, :],
                                    in1=qp_sb[:, b, :],
                                    scale=1.0,
                                    bias=0.0,
                                )
                            else:
                                prod = prodp.tile([128, D_CTX], f16, tag="pr")
                                if route == "ga":
                                    nc.gpsimd.tensor_tensor(
                                        out=prod[:], in0=T4[:, j, :],
                                        in1=qp_sb[:, b, :], op=Alu.mult,
                                    )
                                else:
                                    nc.vector.tensor_tensor(
                                        out=prod[:], in0=T4[:, j, :],
                                        in1=qp_sb[:, b, :], op=Alu.mult,
                                    )
                                nc.scalar.activation(
                                    out=prod[:], in_=prod[:],
                                    func=Act.Copy,
                                    accum_out=scores[:, b, p:p + 1],
                                )

                    s_ch = scores[:, b, c * CH:(c + 1) * CH]
                    e_ch = expw[:, b, c * CH:(c + 1) * CH]
                    zloc = small.tile([128, 1], f32, tag="zl")
                    if c == 0:
                        # fixed exponent frame from chunk 0 (see header)
                        m8 = small.tile([128, 1], f32, tag="m8")
                        nc.vector.tensor_reduce(
                            out=m8[:], in_=s_ch, axis=X, op=Alu.max,
                        )
                        nc.vector.tensor_scalar(
                            negMhat[:, b, :], m8[:], -1.0, -60.0,
                            Alu.mult, Alu.add,
                        )
                        nc.vector.tensor_scalar(
                            clampv[:, b, :], m8[:], 1.0, 140.0,
                            Alu.mult, Alu.add,
                        )
                    else:
                        nc.vector.tensor_scalar(
                            s_ch, s_ch, clampv[:, b, :], None, Alu.min,
                        )
                    nc.scalar.activation(
                        out=e_ch, in_=s_ch, func=Act.Exp,
                        bias=negMhat[:, b, :], scale=1.0,
                        accum_out=zloc[:],
                    )
                    if c == 0:
                        nc.scalar.copy(out=Zrun[:, b, :], in_=zloc[:])
                    else:
                        nc.vector.tensor_tensor(
                            out=Zrun[:, b, :], in0=Zrun[:, b, :],
                            in1=zloc[:], op=Alu.add,
                        )

                    # pooling MACs on PE: diag(e_p) @ T_p, PSUM-accumulated
                    for t in range(TPCH):
                        T4 = t_tiles[t]
                        for j in range(PG):
                            p = (c * TPCH + t) * PG + j
                            e_p = expw[:, b, p:p + 1]
                            dg = diagp.tile([128, BLK], bf16, tag="dg")
                            droute = DIAG_ROUTE[(b, p)]
                            if droute == "v":
                                nc.vector.tensor_scalar(
                                    out=dg[:], in0=eye_sb[:], scalar1=e_p,
                                    scalar2=None, op0=Alu.mult,
                                )
                            else:
                                nc.scalar.activation(
                                    out=dg[:], in_=eye_sb[:], func=Act.Copy,
                                    scale=e_p,
                                )
                            for h in range(2):
                                nc.tensor.matmul(
                                    U[b][h][:],
                                    dg[:],
                                    T4[:, j, h * EH:(h + 1) * EH],
                                    start=(p == 0),
                                    stop=(p == P - 1),
                                    skip_group_check=True,
                                )

                # ---- finalize block: ctx = U / Z -> f32 out ----------------
                rinv = small.tile([128, 1], f32, tag="ri")
                nc.vector.reciprocal(out=rinv[:], in_=Zrun[:, b, :])
                for h in range(2):
                    nc.scalar.activation(
                        out=outsb[:, b, h * EH:(h + 1) * EH],
                        in_=U[b][h][:], func=Act.Copy, scale=rinv[:],
                    )
                nc.sync.dma_start(
                    out=out_d[b * BLK:(b + 1) * BLK, :], in_=outsb[:, b, :]
                )

    nc.compile()
    return nc


def _get_nc():
    global _NC
    if _NC is None:
        _NC = _build()
    return _NC


def _make_in_maps(sent_vecs, proto_vecs, Wq, bq, Wk):
    sent = np.asarray(sent_vecs, dtype=np.float32).reshape(TOK, D_SENT)
    sentT = np.ascontiguousarray(sent.T).astype(np.float16)
    proto = np.asarray(proto_vecs, dtype=np.float32).reshape(TOK, P, D_CTX)
    proto = np.ascontiguousarray(proto.astype(np.float16))
    wq = np.asarray(Wq, dtype=np.float32)
    bq = np.asarray(bq, dtype=np.float32).reshape(1, D_CTX)
    wk = np.asarray(Wk, dtype=np.float32)
    # fold the projection weights host-side: qp = sent @ W + bp
    w = np.ascontiguousarray((wq @ wk.T).astype(np.float16))
    bp = np.ascontiguousarray((bq @ wk.T).astype(np.float16))
    eye = np.eye(BLK, dtype=np.float32)
    import ml_dtypes
    eye = eye.astype(ml_dtypes.bfloat16)
    in_maps = []
    for i in range(N_CORES):
        sl = slice(i * TPC, (i + 1) * TPC)
        in_maps.append(
            {
                "sentT": np.ascontiguousarray(sentT[:, sl]),
                "proto": np.ascontiguousarray(proto[sl]),
                "w": w,
                "bp": bp,
                "eye": eye,
            }
        )
    return in_maps


def _ensure_ntff_hook():
    """The agent image's antenv lacks axon_hooks; shim it so trace=True
    can capture NTFF profiles via the libaxon ctypes path."""
    try:
        from antenv.axon_hooks import get_axon_ntff_profile_hook  # noqa: F401
        return
    except ImportError:
        pass
    import types

    import antenv
    from trn_agent_boot.trn_boot import _ntff_profile_via_ctypes

    mod = types.ModuleType("antenv.axon_hooks")
    mod._hook = _ntff_profile_via_ctypes("/opt/axon/libaxon_pjrt.so")
    mod.get_axon_ntff_profile_hook = lambda: mod._hook
    mod.set_axon_ntff_profile_hook = lambda h: setattr(mod, "_hook", h)
    sys.modules["antenv.axon_hooks"] = mod
    antenv.axon_hooks = mod


def run(sent_vecs, proto_vecs, Wq, bq, Wk, bk=None, trace=False, **kw):
    """Returns (out[4,512,768] float32, BassKernelResults)."""
    from concourse.bass_utils import run_bass_kernel_spmd

    if trace:
        _ensure_ntff_hook()
    nc = _get_nc()
    in_maps = _make_in_maps(sent_vecs, proto_vecs, Wq, bq, Wk)
    res = run_bass_kernel_spmd(
        nc, in_maps, core_ids=list(range(N_CORES)), trace=trace
    )
    outs = [np.asarray(res.results[i]["out"]) for i in range(N_CORES)]
    full = np.concatenate(outs, axis=0).reshape(B, S, D_CTX).astype(np.float32)
    return full, res


def kernel(sent_vecs, proto_vecs, Wq, bq, Wk, bk=None, **kw):
    out, _ = run(sent_vecs, proto_vecs, Wq, bq, Wk, bk)
    return out


if __name__ == "__main__":
    nc = _get_nc()
    print("build + compile OK")
    from concourse.timeline_sim import TimelineSim
    print("TimelineSim predicted:", TimelineSim(nc).simulate(), "ns")


# revision 8
# speedup vs baseline: 1.7752x; 1.1892x over previous
"""AttentiveProtoFusion kernel for 8 TRN2 NeuronCores (v3).

Math (identical algebra to the reference, ~14x fewer FLOPs):
    qp = sent @ (Wq @ Wk^T) + bq @ Wk^T              [n, 768]
    scores[n,p] = sum_c proto[n,p,c] * qp[n,c]   (+ qp.bk const -> dropped)
    w = softmax(scores, axis=p)
    ctx[n,c] = sum_p w[n,p] * proto[n,p,c]

Design (HW-microbenchmarked per-op costs in brackets):
  * proto / sent / W staged host-side in fp16 (same values, half the DMA:
    ~15.5 MB/core -> ~41us DMA roofline; fp16 end-to-end numpy sim gives
    rel err 2.0e-3 vs the 2e-2 gate; bf16 would be 1.6e-2 - too close).
  * Pooling U[n,c] += e[n,p]*T_p[n,c] runs on the TensorEngine as a
    matmul with DIAGONAL stationary diag(e[:,p]) (bf16, built by one
    4x-mode tensor_scalar from a staged identity [167ns]), accumulated
    in PSUM over all 32 protos [163ns matmul + 97ns ldweights per half].
    out[i,j] = sum_k diag[k,i]*T[k,j] = e[i]*T[i,j].
  * Scores are inherently batched per-token dots (no matmul form):
    48 units on DVE affine_mul_reduce (in0 fp16 proto, in1 MUST be f32
    qp - fp16 in1 measures 1412ns vs 868ns) + 8 protos on GPSIMD
    (dual-block tensor_tensor product [~4.2us] + ACT accum-reduce
    [605+185ns]).  Diag builds split DVE/ACT to balance.
  * Proto tiles carry BOTH 128-token blocks of a proto pair
    ([128, 2, 2, 768], rearranged host-side order preserved via a
    strided DMA), so block 1 work is not serialized behind block 0.
  * Online softmax with fixed exponent frame from chunk 0 (Mhat =
    max(chunk0)+60, scores clamped at Mhat+80): exp(s-Mhat) <= e^80
    fits f32/bf16; U/Z equals softmax exactly. Chunks of [12,12,6,2]
    protos - the tiny last chunk shrinks the critical-path tail.

Sharding: data-parallel over the 2048 tokens (B*S), 256 tokens/core.
"""

import sys

for _p in ("/opt/trn_rl_repo", "/opt/pypackages"):
    if _p not in sys.path:
        sys.path.append(_p)

import numpy as np

B, S, P, D_SENT, D_CTX = 4, 512, 32, 1024, 768
N_CORES = 8
TOK = B * S                    # 2048
TPC = TOK // N_CORES           # 256 tokens per core
BLK = 128                      # tokens per block
NBLK = TPC // BLK              # 2
PG = 2                         # prototypes per DMA tile
NPG = P // PG                  # 16 proto tiles (each carries both blocks)
CHUNK_TILES = [6, 6, 3, 1]     # tiles per online chunk -> protos [12,12,6,2]
NCH = len(CHUNK_TILES)
EH = D_CTX // 2                # 384 (psum-bank-sized half of d_ctx)
DS = D_SENT // 128             # 8 contraction chunks for the projection

# protos whose score product runs on GPSIMD (dual-block tensor_tensor);
# the rest run per-block on DVE affine_mul_reduce.
GPS_P = {1, 4, 7, 13, 16, 19, 25, 28}
# diag(e_p) builder: even p on DVE tensor_scalar, odd p on ACT copy-scale
DIAG_ON_ACT = {p for p in range(P) if p % 2 == 1}

_NC = None


def _build():
    import concourse.bass as bass
    import concourse.tile as tile
    from concourse import bacc, mybir

    f32 = mybir.dt.float32
    f16 = mybir.dt.float16
    bf16 = mybir.dt.bfloat16
    Alu = mybir.AluOpType
    Act = mybir.ActivationFunctionType
    X = mybir.AxisListType.X

    nc = bacc.Bacc("TRN2", target_bir_lowering=False)

    sentT_d = nc.dram_tensor("sentT", [D_SENT, TPC], f16, kind="ExternalInput")
    proto_d = nc.dram_tensor("proto", [TPC, P, D_CTX], f16, kind="ExternalInput")
    w_d = nc.dram_tensor("w", [D_SENT, D_CTX], f16, kind="ExternalInput")
    bp_d = nc.dram_tensor("bp", [1, D_CTX], f16, kind="ExternalInput")
    eye_d = nc.dram_tensor("eye", [BLK, BLK], bf16, kind="ExternalInput")
    out_d = nc.dram_tensor("out", [TPC, D_CTX], f32, kind="ExternalOutput")

    with tile.TileContext(nc) as tc:
        with (
            tc.tile_pool(name="wpool", bufs=1) as wpool,
            tc.tile_pool(name="persist", bufs=1) as persist,
            tc.tile_pool(name="ppool", bufs=NPG) as ppool,
            tc.tile_pool(name="prodp", bufs=6) as prodp,
            tc.tile_pool(name="gprodp", bufs=3) as gprodp,
            tc.tile_pool(name="diagp", bufs=8) as diagp,
            tc.tile_pool(name="small", bufs=4) as small,
            tc.tile_pool(name="psq", bufs=2, space="PSUM") as psq,
            tc.tile_pool(name="psu", bufs=1, space="PSUM") as psu,
        ):
            # ---- weights / staged constants --------------------------------
            sentT_sb = wpool.tile([128, DS, TPC], f16)
            nc.sync.dma_start(
                out=sentT_sb[:],
                in_=sentT_d[:].rearrange("(dd p) n -> p dd n", p=128),
            )
            w_sb = wpool.tile([128, DS, D_CTX], f16)
            nc.sync.dma_start(
                out=w_sb[:], in_=w_d[:].rearrange("(dd p) e -> p dd e", p=128)
            )
            bp_sb = wpool.tile([1, D_CTX], f16)
            nc.sync.dma_start(out=bp_sb[:], in_=bp_d[:])
            eye_sb = wpool.tile([128, BLK], bf16)
            nc.sync.dma_start(out=eye_sb[:], in_=eye_d[:])
            ones_sb = wpool.tile([1, 128], f16)
            nc.vector.memset(ones_sb[:], 1.0)

            qp_sb = persist.tile([128, NBLK, D_CTX], f32)
            scores = persist.tile([128, NBLK, P], f32)
            expw = persist.tile([128, NBLK, P], f32)
            negMhat = persist.tile([128, NBLK, 1], f32)
            clampv = persist.tile([128, NBLK, 1], f32)
            zparts = persist.tile([128, NBLK, NCH], f32)
            outsb = persist.tile([128, NBLK, D_CTX], f32)

            # ---- projection: qp = sent @ W + bp  (PE fp16, f32 out) --------
            for b in range(NBLK):
                for h in range(2):
                    pp = psq.tile([128, EH], f32, tag="mm")
                    for dd in range(DS):
                        nc.tensor.matmul(
                            pp[:],
                            sentT_sb[:, dd, b * BLK:(b + 1) * BLK],
                            w_sb[:, dd, h * EH:(h + 1) * EH],
                            start=(dd == 0),
                            stop=False,
                        )
                    nc.tensor.matmul(
                        pp[:],
                        ones_sb[0:1, :],
                        bp_sb[0:1, h * EH:(h + 1) * EH],
                        start=False,
                        stop=True,
                    )
                    nc.scalar.activation(
                        out=qp_sb[:, b, h * EH:(h + 1) * EH], in_=pp[:],
                        func=Act.Copy,
                    )

            # persistent PSUM accumulators for the pooled context
            U = []
            for b in range(NBLK):
                row = []
                for h in range(2):
                    ubh = psu.tile([128, EH], f32, tag=f"U{b}{h}", name=f"U{b}{h}")
                    row.append(ubh)
                U.append(row)

            # ---- main loop: stream protos; scores -> exp -> PE pooling -----
            # proto DRAM is [256, 32, 768]; a tile carries proto pair g for
            # BOTH 128-token blocks: [n, bb, p, e]
            proto_v = proto_d[:].rearrange("(bb n) p e -> n bb p e", n=BLK)
            g0 = 0
            for c, ntiles in enumerate(CHUNK_TILES):
                cp0, cp1 = g0 * PG, (g0 + ntiles) * PG   # proto range of chunk
                t_tiles = []
                for t in range(ntiles):
                    g = g0 + t
                    T2 = ppool.tile([128, NBLK, PG, D_CTX], f16, tag="T")
                    nc.sync.dma_start(
                        out=T2[:], in_=proto_v[:, :, g * PG:(g + 1) * PG, :]
                    )
                    t_tiles.append(T2)
                    for j in range(PG):
                        p = g * PG + j
                        if p in GPS_P:
                            gp = gprodp.tile([128, NBLK, D_CTX], f16, tag="gp")
                            nc.gpsimd.tensor_tensor(
                                out=gp[:], in0=T2[:, :, j, :], in1=qp_sb[:],
                                op=Alu.mult,
                            )
                            for b in range(NBLK):
                                nc.scalar.activation(
                                    out=gp[:, b, :], in_=gp[:, b, :],
                                    func=Act.Copy,
                                    accum_out=scores[:, b, p:p + 1],
                                )
                        else:
                            for b in range(NBLK):
                                prod = prodp.tile([128, D_CTX], f16, tag="pr")
                                nc.vector.affine_mul_reduce(
                                    out=prod[:],
                                    accum_out=scores[:, b, p:p + 1],
                                    in0=T2[:, b, j, :],
                                    in1=qp_sb[:, b, :],
                                    scale=1.0,
                                    bias=0.0,
                                )

                for b in range(NBLK):
                    s_ch = scores[:, b, cp0:cp1]
                    e_ch = expw[:, b, cp0:cp1]
                    if c == 0:
                        # fixed exponent frame from chunk 0 (see header)
                        m0 = small.tile([128, 1], f32, tag="m0")
                        nc.vector.tensor_reduce(
                            out=m0[:], in_=s_ch, axis=X, op=Alu.max,
                        )
                        nc.vector.tensor_scalar(
                            negMhat[:, b, :], m0[:], -1.0, -60.0,
                            Alu.mult, Alu.add,
                        )
                        nc.vector.tensor_scalar(
                            clampv[:, b, :], m0[:], 1.0, 140.0,
                            Alu.mult, Alu.add,
                        )
                    else:
                        nc.vector.tensor_scalar(
                            s_ch, s_ch, clampv[:, b, :], None, Alu.min,
                        )
                    nc.scalar.activation(
                        out=e_ch, in_=s_ch, func=Act.Exp,
                        bias=negMhat[:, b, :], scale=1.0,
                        accum_out=zparts[:, b, c:c + 1],
                    )

                # pooling MACs on PE: diag(e_p) @ T_p, PSUM-accumulated
                for t in range(ntiles):
                    T2 = t_tiles[t]
                    for j in range(PG):
                        p = (g0 + t) * PG + j
                        for b in range(NBLK):
                            e_p = expw[:, b, p:p + 1]
                            dg = diagp.tile([128, BLK], bf16, tag="dg")
                            if p in DIAG_ON_ACT:
                                nc.scalar.activation(
                                    out=dg[:], in_=eye_sb[:], func=Act.Copy,
                                    scale=e_p,
                                )
                            else:
                                nc.vector.tensor_scalar(
                                    out=dg[:], in0=eye_sb[:], scalar1=e_p,
                                    scalar2=None, op0=Alu.mult,
                                )
                            for h in range(2):
                                nc.tensor.matmul(
                                    U[b][h][:],
                                    dg[:],
                                    T2[:, b, j, h * EH:(h + 1) * EH],
                                    start=(p == 0),
                                    stop=(p == P - 1),
                                    skip_group_check=True,
                                )
                g0 += ntiles

            # ---- finalize: ctx = U / Z -> f32 out --------------------------
            for b in range(NBLK):
                zsum = small.tile([128, 1], f32, tag="zs")
                nc.vector.tensor_reduce(
                    out=zsum[:], in_=zparts[:, b, :], axis=X, op=Alu.add,
                )
                rinv = small.tile([128, 1], f32, tag="ri")
                nc.vector.reciprocal(out=rinv[:], in_=zsum[:])
                for h in range(2):
                    nc.scalar.activation(
                        out=outsb[:, b, h * EH:(h + 1) * EH],
                        in_=U[b][h][:], func=Act.Copy, scale=rinv[:],
                    )
                nc.sync.dma_start(
                    out=out_d[b * BLK:(b + 1) * BLK, :], in_=outsb[:, b, :]
                )

    nc.compile()
    return nc


def _get_nc():
    global _NC
    if _NC is None:
        _NC = _build()
    return _NC


def _make_in_maps(sent_vecs, proto_vecs, Wq, bq, Wk):
    sent = np.asarray(sent_vecs, dtype=np.float32).reshape(TOK, D_SENT)
    sentT = np.ascontiguousarray(sent.T).astype(np.float16)
    proto = np.asarray(proto_vecs, dtype=np.float32).reshape(TOK, P, D_CTX)
    proto = np.ascontiguousarray(proto.astype(np.float16))
    wq = np.asarray(Wq, dtype=np.float32)
    bq = np.asarray(bq, dtype=np.float32).reshape(1, D_CTX)
    wk = np.asarray(Wk, dtype=np.float32)
    # fold the projection weights host-side: qp = sent @ W + bp
    w = np.ascontiguousarray((wq @ wk.T).astype(np.float16))
    bp = np.ascontiguousarray((bq @ wk.T).astype(np.float16))
    import ml_dtypes
    eye = np.eye(BLK, dtype=np.float32).astype(ml_dtypes.bfloat16)
    in_maps = []
    for i in range(N_CORES):
        sl = slice(i * TPC, (i + 1) * TPC)
        in_maps.append(
            {
                "sentT": np.ascontiguousarray(sentT[:, sl]),
                "proto": np.ascontiguousarray(proto[sl]),
                "w": w,
                "bp": bp,
                "eye": eye,
            }
        )
    return in_maps


def _ensure_ntff_hook():
    """The agent image's antenv lacks axon_hooks; shim it so trace=True
    can capture NTFF profiles via the libaxon ctypes path."""
    try:
        from antenv.axon_hooks import get_axon_ntff_profile_hook  # noqa: F401
        return
    except ImportError:
        pass
    import types

    import antenv
    from trn_agent_boot.trn_boot import _ntff_profile_via_ctypes

    mod = types.ModuleType("antenv.axon_hooks")
    mod._hook = _ntff_profile_via_ctypes("/opt/axon/libaxon_pjrt.so")
    mod.get_axon_ntff_profile_hook = lambda: mod._hook
    mod.set_axon_ntff_profile_hook = lambda h: setattr(mod, "_hook", h)
    sys.modules["antenv.axon_hooks"] = mod
    antenv.axon_hooks = mod


def run(sent_vecs, proto_vecs, Wq, bq, Wk, bk=None, trace=False, **kw):
    """Returns (out[4,512,768] float32, BassKernelResults)."""
    from concourse.bass_utils import run_bass_kernel_spmd

    if trace:
        _ensure_ntff_hook()
    nc = _get_nc()
    in_maps = _make_in_maps(sent_vecs, proto_vecs, Wq, bq, Wk)
    res = run_bass_kernel_spmd(
        nc, in_maps, core_ids=list(range(N_CORES)), trace=trace
    )
    outs = [np.asarray(res.results[i]["out"]) for i in range(N_CORES)]
    full = np.concatenate(outs, axis=0).reshape(B, S, D_CTX).astype(np.float32)
    return full, res


def kernel(sent_vecs, proto_vecs, Wq, bq, Wk, bk=None, **kw):
    out, _ = run(sent_vecs, proto_vecs, Wq, bq, Wk, bk)
    return out


if __name__ == "__main__":
    nc = _get_nc()
    print("build + compile OK")
    from concourse.timeline_sim import TimelineSim
    print("TimelineSim predicted:", TimelineSim(nc).simulate(), "ns")


# revision 9
# speedup vs baseline: 2.0568x; 1.1586x over previous
"""AttentiveProtoFusion kernel for 8 TRN2 NeuronCores (v3).

Math (identical algebra to the reference, ~14x fewer FLOPs):
    qp = sent @ (Wq @ Wk^T) + bq @ Wk^T              [n, 768]
    scores[n,p] = sum_c proto[n,p,c] * qp[n,c]   (+ qp.bk const -> dropped)
    w = softmax(scores, axis=p)
    ctx[n,c] = sum_p w[n,p] * proto[n,p,c]

Design (HW-microbenchmarked per-op costs in brackets):
  * proto / sent / W staged host-side in fp16 (same values, half the DMA:
    ~15.5 MB/core -> ~41us DMA roofline; fp16 end-to-end numpy sim gives
    rel err 2.0e-3 vs the 2e-2 gate; bf16 would be 1.6e-2 - too close).
  * Pooling U[n,c] += e[n,p]*T_p[n,c] runs on the TensorEngine as a
    matmul with DIAGONAL stationary diag(e[:,p]) (bf16, built by one
    4x-mode tensor_scalar from a staged identity [167ns]), accumulated
    in PSUM over all 32 protos [163ns matmul + 97ns ldweights per half].
    out[i,j] = sum_k diag[k,i]*T[k,j] = e[i]*T[i,j].
  * Scores are inherently batched per-token dots (no matmul form):
    48 units on DVE affine_mul_reduce (in0 fp16 proto, in1 MUST be f32
    qp - fp16 in1 measures 1412ns vs 868ns) + 8 protos on GPSIMD
    (dual-block tensor_tensor product [~4.2us] + ACT accum-reduce
    [605+185ns]).  Diag builds split DVE/ACT to balance.
  * Proto tiles carry BOTH 128-token blocks of a proto pair
    ([128, 2, 2, 768], rearranged host-side order preserved via a
    strided DMA), so block 1 work is not serialized behind block 0.
  * Online softmax with fixed exponent frame from chunk 0 (Mhat =
    max(chunk0)+60, scores clamped at Mhat+80): exp(s-Mhat) <= e^80
    fits f32/bf16; U/Z equals softmax exactly. Chunks of [12,12,6,2]
    protos - the tiny last chunk shrinks the critical-path tail.

Sharding: data-parallel over the 2048 tokens (B*S), 256 tokens/core.
"""

import sys

for _p in ("/opt/trn_rl_repo", "/opt/pypackages"):
    if _p not in sys.path:
        sys.path.append(_p)

import numpy as np

B, S, P, D_SENT, D_CTX = 4, 512, 32, 1024, 768
N_CORES = 8
TOK = B * S                    # 2048
TPC = TOK // N_CORES           # 256 tokens per core
BLK = 128                      # tokens per block
NBLK = TPC // BLK              # 2
PG = 2                         # prototypes per DMA tile
NPG = P // PG                  # 16 proto tiles (each carries both blocks)
CHUNK_TILES = [6, 6, 3, 1]     # tiles per online chunk -> protos [12,12,6,2]
NCH = len(CHUNK_TILES)
EH = D_CTX // 2                # 384 (psum-bank-sized half of d_ctx)
DS = D_SENT // 128             # 8 contraction chunks for the projection

# protos whose score product runs on GPSIMD. Empty: GPSIMD tensor_tensor
# measures 2.4ns/elem AND its SBUF-port contention stretches concurrent
# DVE ops ~1.5x (amr med 870ns -> avg 1355ns while GPS runs) - net loss.
GPS_P = set()
# diag(e_p) builder: ACT copy-scale (~490ns) keeps DVE free for scores;
# DVE tensor_scalar (186ns) for any p listed here.
DIAG_ON_ACT = set(range(P))

_NC = None


def _build():
    import concourse.bass as bass
    import concourse.tile as tile
    from concourse import bacc, mybir

    f32 = mybir.dt.float32
    f16 = mybir.dt.float16
    bf16 = mybir.dt.bfloat16
    Alu = mybir.AluOpType
    Act = mybir.ActivationFunctionType
    X = mybir.AxisListType.X

    nc = bacc.Bacc("TRN2", target_bir_lowering=False)

    sentT_d = nc.dram_tensor("sentT", [D_SENT, TPC], f16, kind="ExternalInput")
    proto_d = nc.dram_tensor("proto", [TPC, P, D_CTX], f16, kind="ExternalInput")
    w_d = nc.dram_tensor("w", [D_SENT, D_CTX], f16, kind="ExternalInput")
    bp_d = nc.dram_tensor("bp", [1, D_CTX], f16, kind="ExternalInput")
    eye_d = nc.dram_tensor("eye", [BLK, BLK], bf16, kind="ExternalInput")
    out_d = nc.dram_tensor("out", [TPC, D_CTX], f32, kind="ExternalOutput")

    with tile.TileContext(nc) as tc:
        with (
            tc.tile_pool(name="wpool", bufs=1) as wpool,
            tc.tile_pool(name="persist", bufs=1) as persist,
            tc.tile_pool(name="ppool", bufs=NPG) as ppool,
            tc.tile_pool(name="prodp", bufs=6) as prodp,
            tc.tile_pool(name="gprodp", bufs=3) as gprodp,
            tc.tile_pool(name="diagp", bufs=8) as diagp,
            tc.tile_pool(name="small", bufs=4) as small,
            tc.tile_pool(name="psq", bufs=2, space="PSUM") as psq,
            tc.tile_pool(name="psu", bufs=1, space="PSUM") as psu,
        ):
            # ---- weights / staged constants --------------------------------
            sentT_sb = wpool.tile([128, DS, TPC], f16)
            nc.sync.dma_start(
                out=sentT_sb[:],
                in_=sentT_d[:].rearrange("(dd p) n -> p dd n", p=128),
            )
            w_sb = wpool.tile([128, DS, D_CTX], f16)
            nc.sync.dma_start(
                out=w_sb[:], in_=w_d[:].rearrange("(dd p) e -> p dd e", p=128)
            )
            bp_sb = wpool.tile([1, D_CTX], f16)
            nc.sync.dma_start(out=bp_sb[:], in_=bp_d[:])
            eye_sb = wpool.tile([128, BLK], bf16)
            nc.sync.dma_start(out=eye_sb[:], in_=eye_d[:])
            ones_sb = wpool.tile([1, 128], f16)
            nc.vector.memset(ones_sb[:], 1.0)

            qp_sb = persist.tile([128, NBLK, D_CTX], f32)
            scores = persist.tile([128, NBLK, P], f32)
            expw = persist.tile([128, NBLK, P], f32)
            negMhat = persist.tile([128, NBLK, 1], f32)
            clampv = persist.tile([128, NBLK, 1], f32)
            zparts = persist.tile([128, NBLK, NCH], f32)
            outsb = persist.tile([128, NBLK, D_CTX], f32)

            # ---- projection: qp = sent @ W + bp  (PE fp16, f32 out) --------
            for b in range(NBLK):
                for h in range(2):
                    pp = psq.tile([128, EH], f32, tag="mm")
                    for dd in range(DS):
                        nc.tensor.matmul(
                            pp[:],
                            sentT_sb[:, dd, b * BLK:(b + 1) * BLK],
                            w_sb[:, dd, h * EH:(h + 1) * EH],
                            start=(dd == 0),
                            stop=False,
                        )
                    nc.tensor.matmul(
                        pp[:],
                        ones_sb[0:1, :],
                        bp_sb[0:1, h * EH:(h + 1) * EH],
                        start=False,
                        stop=True,
                    )
                    nc.scalar.activation(
                        out=qp_sb[:, b, h * EH:(h + 1) * EH], in_=pp[:],
                        func=Act.Copy,
                    )

            # persistent PSUM accumulators for the pooled context
            U = []
            for b in range(NBLK):
                row = []
                for h in range(2):
                    ubh = psu.tile([128, EH], f32, tag=f"U{b}{h}", name=f"U{b}{h}")
                    row.append(ubh)
                U.append(row)

            # ---- main loop: stream protos; scores -> exp -> PE pooling -----
            # proto DRAM is [256, 32, 768]; a tile carries proto pair g for
            # BOTH 128-token blocks: [n, bb, p, e]
            proto_v = proto_d[:].rearrange("(bb n) p e -> n bb p e", n=BLK)
            g0 = 0
            for c, ntiles in enumerate(CHUNK_TILES):
                cp0, cp1 = g0 * PG, (g0 + ntiles) * PG   # proto range of chunk
                t_tiles = []
                for t in range(ntiles):
                    g = g0 + t
                    T2 = ppool.tile([128, NBLK, PG, D_CTX], f16, tag="T")
                    nc.sync.dma_start(
                        out=T2[:], in_=proto_v[:, :, g * PG:(g + 1) * PG, :]
                    )
                    t_tiles.append(T2)
                    for j in range(PG):
                        p = g * PG + j
                        if p in GPS_P:
                            gp = gprodp.tile([128, NBLK, D_CTX], f16, tag="gp")
                            nc.gpsimd.tensor_tensor(
                                out=gp[:], in0=T2[:, :, j, :], in1=qp_sb[:],
                                op=Alu.mult,
                            )
                            for b in range(NBLK):
                                nc.scalar.activation(
                                    out=gp[:, b, :], in_=gp[:, b, :],
                                    func=Act.Copy,
                                    accum_out=scores[:, b, p:p + 1],
                                )
                        else:
                            for b in range(NBLK):
                                prod = prodp.tile([128, D_CTX], f16, tag="pr")
                                nc.vector.affine_mul_reduce(
                                    out=prod[:],
                                    accum_out=scores[:, b, p:p + 1],
                                    in0=T2[:, b, j, :],
                                    in1=qp_sb[:, b, :],
                                    scale=1.0,
                                    bias=0.0,
                                )

                for b in range(NBLK):
                    s_ch = scores[:, b, cp0:cp1]
                    e_ch = expw[:, b, cp0:cp1]
                    if c == 0:
                        # fixed exponent frame from chunk 0 (see header)
                        m0 = small.tile([128, 1], f32, tag="m0")
                        nc.vector.tensor_reduce(
                            out=m0[:], in_=s_ch, axis=X, op=Alu.max,
                        )
                        nc.vector.tensor_scalar(
                            negMhat[:, b, :], m0[:], -1.0, -60.0,
                            Alu.mult, Alu.add,
                        )
                        nc.vector.tensor_scalar(
                            clampv[:, b, :], m0[:], 1.0, 140.0,
                            Alu.mult, Alu.add,
                        )
                    else:
                        nc.vector.tensor_scalar(
                            s_ch, s_ch, clampv[:, b, :], None, Alu.min,
                        )
                    nc.scalar.activation(
                        out=e_ch, in_=s_ch, func=Act.Exp,
                        bias=negMhat[:, b, :], scale=1.0,
                        accum_out=zparts[:, b, c:c + 1],
                    )

                # pooling MACs on PE: diag(e_p) @ T_p, PSUM-accumulated
                for t in range(ntiles):
                    T2 = t_tiles[t]
                    for j in range(PG):
                        p = (g0 + t) * PG + j
                        for b in range(NBLK):
                            e_p = expw[:, b, p:p + 1]
                            dg = diagp.tile([128, BLK], bf16, tag="dg")
                            if p in DIAG_ON_ACT:
                                nc.scalar.activation(
                                    out=dg[:], in_=eye_sb[:], func=Act.Copy,
                                    scale=e_p,
                                )
                            else:
                                nc.vector.tensor_scalar(
                                    out=dg[:], in0=eye_sb[:], scalar1=e_p,
                                    scalar2=None, op0=Alu.mult,
                                )
                            for h in range(2):
                                nc.tensor.matmul(
                                    U[b][h][:],
                                    dg[:],
                                    T2[:, b, j, h * EH:(h + 1) * EH],
                                    start=(p == 0),
                                    stop=(p == P - 1),
                                    skip_group_check=True,
                                )
                g0 += ntiles

            # ---- finalize: ctx = U / Z -> f32 out --------------------------
            for b in range(NBLK):
                zsum = small.tile([128, 1], f32, tag="zs")
                nc.vector.tensor_reduce(
                    out=zsum[:], in_=zparts[:, b, :], axis=X, op=Alu.add,
                )
                rinv = small.tile([128, 1], f32, tag="ri")
                nc.vector.reciprocal(out=rinv[:], in_=zsum[:])
                for h in range(2):
                    nc.scalar.activation(
                        out=outsb[:, b, h * EH:(h + 1) * EH],
                        in_=U[b][h][:], func=Act.Copy, scale=rinv[:],
                    )
                nc.sync.dma_start(
                    out=out_d[b * BLK:(b + 1) * BLK, :], in_=outsb[:, b, :]
                )

    nc.compile()
    return nc


def _get_nc():
    global _NC
    if _NC is None:
        _NC = _build()
    return _NC


def _make_in_maps(sent_vecs, proto_vecs, Wq, bq, Wk):
    sent = np.asarray(sent_vecs, dtype=np.float32).reshape(TOK, D_SENT)
    sentT = np.ascontiguousarray(sent.T).astype(np.float16)
    proto = np.asarray(proto_vecs, dtype=np.float32).reshape(TOK, P, D_CTX)
    proto = np.ascontiguousarray(proto.astype(np.float16))
    wq = np.asarray(Wq, dtype=np.float32)
    bq = np.asarray(bq, dtype=np.float32).reshape(1, D_CTX)
    wk = np.asarray(Wk, dtype=np.float32)
    # fold the projection weights host-side: qp = sent @ W + bp
    w = np.ascontiguousarray((wq @ wk.T).astype(np.float16))
    bp = np.ascontiguousarray((bq @ wk.T).astype(np.float16))
    import ml_dtypes
    eye = np.eye(BLK, dtype=np.float32).astype(ml_dtypes.bfloat16)
    in_maps = []
    for i in range(N_CORES):
        sl = slice(i * TPC, (i + 1) * TPC)
        in_maps.append(
            {
                "sentT": np.ascontiguousarray(sentT[:, sl]),
                "proto": np.ascontiguousarray(proto[sl]),
                "w": w,
                "bp": bp,
                "eye": eye,
            }
        )
    return in_maps


def _ensure_ntff_hook():
    """The agent image's antenv lacks axon_hooks; shim it so trace=True
    can capture NTFF profiles via the libaxon ctypes path."""
    try:
        from antenv.axon_hooks import get_axon_ntff_profile_hook  # noqa: F401
        return
    except ImportError:
        pass
    import types

    import antenv
    from trn_agent_boot.trn_boot import _ntff_profile_via_ctypes

    mod = types.ModuleType("antenv.axon_hooks")
    mod._hook = _ntff_profile_via_ctypes("/opt/axon/libaxon_pjrt.so")
    mod.get_axon_ntff_profile_hook = lambda: mod._hook
    mod.set_axon_ntff_profile_hook = lambda h: setattr(mod, "_hook", h)
    sys.modules["antenv.axon_hooks"] = mod
    antenv.axon_hooks = mod


def run(sent_vecs, proto_vecs, Wq, bq, Wk, bk=None, trace=False, **kw):
    """Returns (out[4,512,768] float32, BassKernelResults)."""
    from concourse.bass_utils import run_bass_kernel_spmd

    if trace:
        _ensure_ntff_hook()
    nc = _get_nc()
    in_maps = _make_in_maps(sent_vecs, proto_vecs, Wq, bq, Wk)
    res = run_bass_kernel_spmd(
        nc, in_maps, core_ids=list(range(N_CORES)), trace=trace
    )
    outs = [np.asarray(res.results[i]["out"]) for i in range(N_CORES)]
    full = np.concatenate(outs, axis=0).reshape(B, S, D_CTX).astype(np.float32)
    return full, res


def kernel(sent_vecs, proto_vecs, Wq, bq, Wk, bk=None, **kw):
    out, _ = run(sent_vecs, proto_vecs, Wq, bq, Wk, bk)
    return out


if __name__ == "__main__":
    nc = _get_nc()
    print("build + compile OK")
    from concourse.timeline_sim import TimelineSim
    print("TimelineSim predicted:", TimelineSim(nc).simulate(), "ns")


# revision 12
# speedup vs baseline: 2.1080x; 1.0249x over previous
"""AttentiveProtoFusion kernel for 8 TRN2 NeuronCores (v3).

Math (identical algebra to the reference, ~14x fewer FLOPs):
    qp = sent @ (Wq @ Wk^T) + bq @ Wk^T              [n, 768]
    scores[n,p] = sum_c proto[n,p,c] * qp[n,c]   (+ qp.bk const -> dropped)
    w = softmax(scores, axis=p)
    ctx[n,c] = sum_p w[n,p] * proto[n,p,c]

Design (HW-microbenchmarked per-op costs in brackets):
  * proto / sent / W staged host-side in fp16 (same values, half the DMA:
    ~15.5 MB/core -> ~41us DMA roofline; fp16 end-to-end numpy sim gives
    rel err 2.0e-3 vs the 2e-2 gate; bf16 would be 1.6e-2 - too close).
  * Pooling U[n,c] += e[n,p]*T_p[n,c] runs on the TensorEngine as a
    matmul with DIAGONAL stationary diag(e[:,p]) (bf16, built by one
    4x-mode tensor_scalar from a staged identity [167ns]), accumulated
    in PSUM over all 32 protos [163ns matmul + 97ns ldweights per half].
    out[i,j] = sum_k diag[k,i]*T[k,j] = e[i]*T[i,j].
  * Scores are inherently batched per-token dots (no matmul form):
    48 units on DVE affine_mul_reduce (in0 fp16 proto, in1 MUST be f32
    qp - fp16 in1 measures 1412ns vs 868ns) + 8 protos on GPSIMD
    (dual-block tensor_tensor product [~4.2us] + ACT accum-reduce
    [605+185ns]).  Diag builds split DVE/ACT to balance.
  * Proto tiles carry BOTH 128-token blocks of a proto pair
    ([128, 2, 2, 768], rearranged host-side order preserved via a
    strided DMA), so block 1 work is not serialized behind block 0.
  * Online softmax with fixed exponent frame from chunk 0 (Mhat =
    max(chunk0)+60, scores clamped at Mhat+80): exp(s-Mhat) <= e^80
    fits f32/bf16; U/Z equals softmax exactly. Chunks of [12,12,6,2]
    protos - the tiny last chunk shrinks the critical-path tail.

Sharding: data-parallel over the 2048 tokens (B*S), 256 tokens/core.
"""

import sys

for _p in ("/opt/trn_rl_repo", "/opt/pypackages"):
    if _p not in sys.path:
        sys.path.append(_p)

import numpy as np

B, S, P, D_SENT, D_CTX = 4, 512, 32, 1024, 768
N_CORES = 8
TOK = B * S                    # 2048
TPC = TOK // N_CORES           # 256 tokens per core
BLK = 128                      # tokens per block
NBLK = TPC // BLK              # 2
PG = 2                         # prototypes per DMA tile
NPG = P // PG                  # 16 proto tiles (each carries both blocks)
CHUNK_TILES = [6, 6, 3, 1]     # tiles per online chunk -> protos [12,12,6,2]
NCH = len(CHUNK_TILES)
EH = D_CTX // 2                # 384 (psum-bank-sized half of d_ctx)
DS = D_SENT // 128             # 8 contraction chunks for the projection

# protos whose score product runs on GPSIMD. Empty: GPSIMD tensor_tensor
# measures 2.4ns/elem AND its SBUF-port contention stretches concurrent
# DVE ops ~1.5x (amr med 870ns -> avg 1355ns while GPS runs) - net loss.
GPS_P = set()
# diag(e_p) builder: ACT copy-scale (~490ns) keeps DVE free while amr
# streams; the last two chunks' diags go on DVE (186ns) because by then
# DVE is drained and the chunk tail is latency-critical.
DIAG_ON_ACT = set(range(24))

_NC = None


def _build():
    import concourse.bass as bass
    import concourse.tile as tile
    from concourse import bacc, mybir

    f32 = mybir.dt.float32
    f16 = mybir.dt.float16
    bf16 = mybir.dt.bfloat16
    Alu = mybir.AluOpType
    Act = mybir.ActivationFunctionType
    X = mybir.AxisListType.X

    nc = bacc.Bacc("TRN2", target_bir_lowering=False)

    sentT_d = nc.dram_tensor("sentT", [D_SENT, TPC], f16, kind="ExternalInput")
    proto_d = nc.dram_tensor("proto", [TPC, P, D_CTX], f16, kind="ExternalInput")
    w_d = nc.dram_tensor("w", [D_SENT, D_CTX], f16, kind="ExternalInput")
    bp_d = nc.dram_tensor("bp", [1, D_CTX], f16, kind="ExternalInput")
    eye_d = nc.dram_tensor("eye", [BLK, BLK], bf16, kind="ExternalInput")
    out_d = nc.dram_tensor("out", [TPC, D_CTX], f32, kind="ExternalOutput")

    with tile.TileContext(nc) as tc:
        with (
            tc.tile_pool(name="wpool", bufs=1) as wpool,
            tc.tile_pool(name="persist", bufs=1) as persist,
            tc.tile_pool(name="ppool", bufs=NPG) as ppool,
            tc.tile_pool(name="prodp", bufs=6) as prodp,
            tc.tile_pool(name="gprodp", bufs=3) as gprodp,
            tc.tile_pool(name="diagp", bufs=8) as diagp,
            tc.tile_pool(name="small", bufs=4) as small,
            tc.tile_pool(name="psq", bufs=2, space="PSUM") as psq,
            tc.tile_pool(name="psu", bufs=1, space="PSUM") as psu,
        ):
            # ---- weights / staged constants --------------------------------
            # One DMA per contraction chunk so the projection matmuls start
            # as soon as their slice lands instead of after the whole 2.1 MB.
            sentT_sb = wpool.tile([128, DS, TPC], f16)
            sentT_v = sentT_d[:].rearrange("(dd p) n -> p dd n", p=128)
            nc.sync.dma_start(out=sentT_sb[:], in_=sentT_v)
            bp_sb = wpool.tile([1, D_CTX], f16)
            nc.sync.dma_start(out=bp_sb[:], in_=bp_d[:])
            eye_sb = wpool.tile([128, BLK], bf16)
            nc.sync.dma_start(out=eye_sb[:], in_=eye_d[:])
            w_sb = wpool.tile([128, DS, D_CTX], f16)
            w_v = w_d[:].rearrange("(dd p) e -> p dd e", p=128)
            for dd in range(DS):
                nc.sync.dma_start(
                    out=w_sb[:, dd, :], in_=w_v[:, dd, :]
                )
            ones_sb = wpool.tile([1, 128], f16)
            nc.vector.memset(ones_sb[:], 1.0)

            qp_sb = persist.tile([128, NBLK, D_CTX], f32)
            scores = persist.tile([128, NBLK, P], f32)
            expw = persist.tile([128, NBLK, P], f32)
            negMhat = persist.tile([128, NBLK, 1], f32)
            clampv = persist.tile([128, NBLK, 1], f32)
            zparts = persist.tile([128, NBLK, NCH], f32)
            outsb = persist.tile([128, NBLK, D_CTX], f32)

            # ---- projection: qp = sent @ W + bp  (PE fp16, f32 out) --------
            for b in range(NBLK):
                for h in range(2):
                    pp = psq.tile([128, EH], f32, tag="mm")
                    for dd in range(DS):
                        nc.tensor.matmul(
                            pp[:],
                            sentT_sb[:, dd, b * BLK:(b + 1) * BLK],
                            w_sb[:, dd, h * EH:(h + 1) * EH],
                            start=(dd == 0),
                            stop=False,
                        )
                    nc.tensor.matmul(
                        pp[:],
                        ones_sb[0:1, :],
                        bp_sb[0:1, h * EH:(h + 1) * EH],
                        start=False,
                        stop=True,
                    )
                    nc.scalar.activation(
                        out=qp_sb[:, b, h * EH:(h + 1) * EH], in_=pp[:],
                        func=Act.Copy,
                    )

            # persistent PSUM accumulators for the pooled context
            U = []
            for b in range(NBLK):
                row = []
                for h in range(2):
                    ubh = psu.tile([128, EH], f32, tag=f"U{b}{h}", name=f"U{b}{h}")
                    row.append(ubh)
                U.append(row)

            # ---- main loop: stream protos; scores -> exp -> PE pooling -----
            # proto DRAM is [256, 32, 768]; a tile carries proto pair g for
            # BOTH 128-token blocks: [n, bb, p, e]
            proto_v = proto_d[:].rearrange("(bb n) p e -> n bb p e", n=BLK)
            g0 = 0
            for c, ntiles in enumerate(CHUNK_TILES):
                cp0, cp1 = g0 * PG, (g0 + ntiles) * PG   # proto range of chunk
                t_tiles = []
                for t in range(ntiles):
                    g = g0 + t
                    T2 = ppool.tile([128, NBLK, PG, D_CTX], f16, tag="T")
                    nc.sync.dma_start(
                        out=T2[:], in_=proto_v[:, :, g * PG:(g + 1) * PG, :]
                    )
                    t_tiles.append(T2)
                    for j in range(PG):
                        p = g * PG + j
                        if p in GPS_P:
                            gp = gprodp.tile([128, NBLK, D_CTX], f16, tag="gp")
                            nc.gpsimd.tensor_tensor(
                                out=gp[:], in0=T2[:, :, j, :], in1=qp_sb[:],
                                op=Alu.mult,
                            )
                            for b in range(NBLK):
                                nc.scalar.activation(
                                    out=gp[:, b, :], in_=gp[:, b, :],
                                    func=Act.Copy,
                                    accum_out=scores[:, b, p:p + 1],
                                )
                        else:
                            for b in range(NBLK):
                                prod = prodp.tile([128, D_CTX], f16, tag="pr")
                                nc.vector.affine_mul_reduce(
                                    out=prod[:],
                                    accum_out=scores[:, b, p:p + 1],
                                    in0=T2[:, b, j, :],
                                    in1=qp_sb[:, b, :],
                                    scale=1.0,
                                    bias=0.0,
                                )

                for b in range(NBLK):
                    s_ch = scores[:, b, cp0:cp1]
                    e_ch = expw[:, b, cp0:cp1]
                    if c == 0:
                        # fixed exponent frame from chunk 0 (see header)
                        m0 = small.tile([128, 1], f32, tag="m0")
                        nc.vector.tensor_reduce(
                            out=m0[:], in_=s_ch, axis=X, op=Alu.max,
                        )
                        nc.vector.tensor_scalar(
                            negMhat[:, b, :], m0[:], -1.0, -60.0,
                            Alu.mult, Alu.add,
                        )
                        nc.vector.tensor_scalar(
                            clampv[:, b, :], m0[:], 1.0, 140.0,
                            Alu.mult, Alu.add,
                        )
                    else:
                        nc.vector.tensor_scalar(
                            s_ch, s_ch, clampv[:, b, :], None, Alu.min,
                        )
                    nc.scalar.activation(
                        out=e_ch, in_=s_ch, func=Act.Exp,
                        bias=negMhat[:, b, :], scale=1.0,
                        accum_out=zparts[:, b, c:c + 1],
                    )

                # pooling MACs on PE: diag(e_p) @ T_p, PSUM-accumulated
                for t in range(ntiles):
                    T2 = t_tiles[t]
                    for j in range(PG):
                        p = (g0 + t) * PG + j
                        for b in range(NBLK):
                            e_p = expw[:, b, p:p + 1]
                            dg = diagp.tile([128, BLK], bf16, tag="dg")
                            if p in DIAG_ON_ACT:
                                nc.scalar.activation(
                                    out=dg[:], in_=eye_sb[:], func=Act.Copy,
                                    scale=e_p,
                                )
                            else:
                                nc.vector.tensor_scalar(
                                    out=dg[:], in0=eye_sb[:], scalar1=e_p,
                                    scalar2=None, op0=Alu.mult,
                                )
                            for h in range(2):
                                nc.tensor.matmul(
                                    U[b][h][:],
                                    dg[:],
                                    T2[:, b, j, h * EH:(h + 1) * EH],
                                    start=(p == 0),
                                    stop=(p == P - 1),
                                    skip_group_check=True,
                                )
                g0 += ntiles

            # ---- finalize: ctx = U / Z -> f32 out --------------------------
            for b in range(NBLK):
                zsum = small.tile([128, 1], f32, tag="zs")
                nc.vector.tensor_reduce(
                    out=zsum[:], in_=zparts[:, b, :], axis=X, op=Alu.add,
                )
                rinv = small.tile([128, 1], f32, tag="ri")
                nc.vector.reciprocal(out=rinv[:], in_=zsum[:])
                for h in range(2):
                    nc.scalar.activation(
                        out=outsb[:, b, h * EH:(h + 1) * EH],
                        in_=U[b][h][:], func=Act.Copy, scale=rinv[:],
                    )
                    nc.sync.dma_start(
                        out=out_d[b * BLK:(b + 1) * BLK, h * EH:(h + 1) * EH],
                        in_=outsb[:, b, h * EH:(h + 1) * EH],
                    )

    nc.compile()
    return nc


def _get_nc():
    global _NC
    if _NC is None:
        _NC = _build()
    return _NC


def _make_in_maps(sent_vecs, proto_vecs, Wq, bq, Wk):
    sent = np.asarray(sent_vecs, dtype=np.float32).reshape(TOK, D_SENT)
    sentT = np.ascontiguousarray(sent.T).astype(np.float16)
    proto = np.asarray(proto_vecs, dtype=np.float32).reshape(TOK, P, D_CTX)
    proto = np.ascontiguousarray(proto.astype(np.float16))
    wq = np.asarray(Wq, dtype=np.float32)
    bq = np.asarray(bq, dtype=np.float32).reshape(1, D_CTX)
    wk = np.asarray(Wk, dtype=np.float32)
    # fold the projection weights host-side: qp = sent @ W + bp
    w = np.ascontiguousarray((wq @ wk.T).astype(np.float16))
    bp = np.ascontiguousarray((bq @ wk.T).astype(np.float16))
    import ml_dtypes
    eye = np.eye(BLK, dtype=np.float32).astype(ml_dtypes.bfloat16)
    in_maps = []
    for i in range(N_CORES):
        sl = slice(i * TPC, (i + 1) * TPC)
        in_maps.append(
            {
                "sentT": np.ascontiguousarray(sentT[:, sl]),
                "proto": np.ascontiguousarray(proto[sl]),
                "w": w,
                "bp": bp,
                "eye": eye,
            }
        )
    return in_maps


def _ensure_ntff_hook():
    """The agent image's antenv lacks axon_hooks; shim it so trace=True
    can capture NTFF profiles via the libaxon ctypes path."""
    try:
        from antenv.axon_hooks import get_axon_ntff_profile_hook  # noqa: F401
        return
    except ImportError:
        pass
    import types

    import antenv
    from trn_agent_boot.trn_boot import _ntff_profile_via_ctypes

    mod = types.ModuleType("antenv.axon_hooks")
    mod._hook = _ntff_profile_via_ctypes("/opt/axon/libaxon_pjrt.so")
    mod.get_axon_ntff_profile_hook = lambda: mod._hook
    mod.set_axon_ntff_profile_hook = lambda h: setattr(mod, "_hook", h)
    sys.modules["antenv.axon_hooks"] = mod
    antenv.axon_hooks = mod


def run(sent_vecs, proto_vecs, Wq, bq, Wk, bk=None, trace=False, **kw):
    """Returns (out[4,512,768] float32, BassKernelResults)."""
    from concourse.bass_utils import run_bass_kernel_spmd

    if trace:
        _ensure_ntff_hook()
    nc = _get_nc()
    in_maps = _make_in_maps(sent_vecs, proto_vecs, Wq, bq, Wk)
    res = run_bass_kernel_spmd(
        nc, in_maps, core_ids=list(range(N_CORES)), trace=trace
    )
    outs = [np.asarray(res.results[i]["out"]) for i in range(N_CORES)]
    full = np.concatenate(outs, axis=0).reshape(B, S, D_CTX).astype(np.float32)
    return full, res


def kernel(sent_vecs, proto_vecs, Wq, bq, Wk, bk=None, **kw):
    out, _ = run(sent_vecs, proto_vecs, Wq, bq, Wk, bk)
    return out


if __name__ == "__main__":
    nc = _get_nc()
    print("build + compile OK")
    from concourse.timeline_sim import TimelineSim
    print("TimelineSim predicted:", TimelineSim(nc).simulate(), "ns")


# revision 18
# speedup vs baseline: 2.1275x; 1.0093x over previous
"""AttentiveProtoFusion kernel for 8 TRN2 NeuronCores (v3).

Math (identical algebra to the reference, ~14x fewer FLOPs):
    qp = sent @ (Wq @ Wk^T) + bq @ Wk^T              [n, 768]
    scores[n,p] = sum_c proto[n,p,c] * qp[n,c]   (+ qp.bk const -> dropped)
    w = softmax(scores, axis=p)
    ctx[n,c] = sum_p w[n,p] * proto[n,p,c]

Design (HW-microbenchmarked per-op costs in brackets):
  * proto / sent / W staged host-side in fp16 (same values, half the DMA:
    ~15.5 MB/core -> ~41us DMA roofline; fp16 end-to-end numpy sim gives
    rel err 2.0e-3 vs the 2e-2 gate; bf16 would be 1.6e-2 - too close).
  * Pooling U[n,c] += e[n,p]*T_p[n,c] runs on the TensorEngine as a
    matmul with DIAGONAL stationary diag(e[:,p]) (bf16, built by one
    4x-mode tensor_scalar from a staged identity [167ns]), accumulated
    in PSUM over all 32 protos [163ns matmul + 97ns ldweights per half].
    out[i,j] = sum_k diag[k,i]*T[k,j] = e[i]*T[i,j].
  * Scores are inherently batched per-token dots (no matmul form):
    48 units on DVE affine_mul_reduce (in0 fp16 proto, in1 MUST be f32
    qp - fp16 in1 measures 1412ns vs 868ns) + 8 protos on GPSIMD
    (dual-block tensor_tensor product [~4.2us] + ACT accum-reduce
    [605+185ns]).  Diag builds split DVE/ACT to balance.
  * Proto tiles carry BOTH 128-token blocks of a proto pair
    ([128, 2, 2, 768], rearranged host-side order preserved via a
    strided DMA), so block 1 work is not serialized behind block 0.
  * Online softmax with fixed exponent frame from chunk 0 (Mhat =
    max(chunk0)+60, scores clamped at Mhat+80): exp(s-Mhat) <= e^80
    fits f32/bf16; U/Z equals softmax exactly. Chunks of [12,12,6,2]
    protos - the tiny last chunk shrinks the critical-path tail.

Sharding: data-parallel over the 2048 tokens (B*S), 256 tokens/core.
"""

import sys

for _p in ("/opt/trn_rl_repo", "/opt/pypackages"):
    if _p not in sys.path:
        sys.path.append(_p)

import numpy as np

B, S, P, D_SENT, D_CTX = 4, 512, 32, 1024, 768
N_CORES = 8
TOK = B * S                    # 2048
TPC = TOK // N_CORES           # 256 tokens per core
BLK = 128                      # tokens per block
NBLK = TPC // BLK              # 2
PG = 2                         # prototypes per DMA tile
NPG = P // PG                  # 16 proto tiles (each carries both blocks)
CHUNK_TILES = [6, 6, 3, 1]     # tiles per online chunk -> protos [12,12,6,2]
NCH = len(CHUNK_TILES)
EH = D_CTX // 2                # 384 (psum-bank-sized half of d_ctx)
DS = D_SENT // 128             # 8 contraction chunks for the projection

# protos whose score product runs on GPSIMD. Empty: GPSIMD tensor_tensor
# measures 2.4ns/elem AND its SBUF-port contention stretches concurrent
# DVE ops ~1.5x (amr med 870ns -> avg 1355ns while GPS runs) - net loss.
GPS_P = set()
# diag(e_p) builder: ACT copy-scale (~490ns) keeps DVE free while amr
# streams; the last two chunks' diags go on DVE (186ns) because by then
# DVE is drained and the chunk tail is latency-critical.
DIAG_ON_ACT = set(range(24))

_NC = None


def _build():
    import concourse.bass as bass
    import concourse.tile as tile
    from concourse import bacc, mybir

    f32 = mybir.dt.float32
    f16 = mybir.dt.float16
    bf16 = mybir.dt.bfloat16
    Alu = mybir.AluOpType
    Act = mybir.ActivationFunctionType
    X = mybir.AxisListType.X

    nc = bacc.Bacc("TRN2", target_bir_lowering=False)

    sentT_d = nc.dram_tensor("sentT", [D_SENT, TPC], f16, kind="ExternalInput")
    proto_d = nc.dram_tensor("proto", [TPC, P, D_CTX], f16, kind="ExternalInput")
    w_d = nc.dram_tensor("w", [D_SENT, D_CTX], f16, kind="ExternalInput")
    bp_d = nc.dram_tensor("bp", [1, D_CTX], f16, kind="ExternalInput")
    eye_d = nc.dram_tensor("eye", [BLK, BLK], bf16, kind="ExternalInput")
    out_d = nc.dram_tensor("out", [TPC, D_CTX], f32, kind="ExternalOutput")

    with tile.TileContext(nc) as tc:
        with (
            tc.tile_pool(name="wpool", bufs=1) as wpool,
            tc.tile_pool(name="persist", bufs=1) as persist,
            tc.tile_pool(name="ppool", bufs=NPG) as ppool,
            tc.tile_pool(name="prodp", bufs=6) as prodp,
            tc.tile_pool(name="gprodp", bufs=3) as gprodp,
            tc.tile_pool(name="diagp", bufs=8) as diagp,
            tc.tile_pool(name="small", bufs=4) as small,
            tc.tile_pool(name="psq", bufs=2, space="PSUM") as psq,
            tc.tile_pool(name="psu", bufs=1, space="PSUM") as psu,
        ):
            # ---- weights / staged constants --------------------------------
            # One DMA per contraction chunk so the projection matmuls start
            # as soon as their slice lands instead of after the whole 2.1 MB.
            sentT_sb = wpool.tile([128, DS, TPC], f16)
            sentT_v = sentT_d[:].rearrange("(dd p) n -> p dd n", p=128)
            nc.sync.dma_start(out=sentT_sb[:], in_=sentT_v)
            bp_sb = wpool.tile([1, D_CTX], f16)
            nc.sync.dma_start(out=bp_sb[:], in_=bp_d[:])
            eye_sb = wpool.tile([128, BLK], bf16)
            nc.sync.dma_start(out=eye_sb[:], in_=eye_d[:])
            w_sb = wpool.tile([128, DS, D_CTX], f16)
            w_v = w_d[:].rearrange("(dd p) e -> p dd e", p=128)
            for dd in range(DS):
                nc.sync.dma_start(
                    out=w_sb[:, dd, :], in_=w_v[:, dd, :]
                )
            ones_sb = wpool.tile([1, 128], f16)
            nc.vector.memset(ones_sb[:], 1.0)

            qp_sb = persist.tile([128, NBLK, D_CTX], f32)
            # per-chunk score/exp tiles: separate tiles avoid false
            # WAR/RAW serialization between chunks on one shared tile
            scores = [persist.tile([128, NBLK, cs * PG], f32, name=f"sc{c}")
                      for c, cs in enumerate(CHUNK_TILES)]
            expw = [persist.tile([128, NBLK, cs * PG], f32, name=f"ew{c}")
                    for c, cs in enumerate(CHUNK_TILES)]
            negMhat = persist.tile([128, NBLK, 1], f32)
            clampv = persist.tile([128, NBLK, 1], f32)
            zparts = persist.tile([128, NBLK, NCH], f32)
            outsb = persist.tile([128, NBLK, D_CTX], f32)

            # ---- engine warm-ups (absorb table/uop loads during DMA) -------
            wu16 = wpool.tile([128, 8], f16)
            wu32 = wpool.tile([128, 8], f32)
            wuac = wpool.tile([128, 1], f32)
            nc.vector.memset(wu16[:], 0.0)
            nc.vector.memset(wu32[:], 0.0)
            nc.vector.affine_mul_reduce(
                out=wu16[:], accum_out=wuac[:], in0=wu16[:], in1=wu32[:],
                scale=1.0, bias=0.0,
            )
            nc.scalar.activation(out=wu32[:], in_=wu32[:], func=Act.Copy)
            # ---- projection: qp = sent @ W + bp  (PE fp16, f32 out) --------
            for b in range(NBLK):
                pps = [psq.tile([128, EH], f32, tag=f"mm{h}", name=f"pp{h}")
                       for h in range(2)]
                for dd in range(DS):
                    for h in range(2):
                        nc.tensor.matmul(
                            pps[h][:],
                            sentT_sb[:, dd, b * BLK:(b + 1) * BLK],
                            w_sb[:, dd, h * EH:(h + 1) * EH],
                            start=(dd == 0),
                            stop=False,
                        )
                for h in range(2):
                    nc.tensor.matmul(
                        pps[h][:],
                        ones_sb[0:1, :],
                        bp_sb[0:1, h * EH:(h + 1) * EH],
                        start=False,
                        stop=True,
                    )
                    nc.scalar.activation(
                        out=qp_sb[:, b, h * EH:(h + 1) * EH], in_=pps[h][:],
                        func=Act.Copy,
                    )

            # persistent PSUM accumulators for the pooled context
            U = []
            for b in range(NBLK):
                row = []
                for h in range(2):
                    ubh = psu.tile([128, EH], f32, tag=f"U{b}{h}", name=f"U{b}{h}")
                    row.append(ubh)
                U.append(row)
            # PE warm-up (pipeline/p-state); overwritten by the real chain's
            # start=True matmul later
            nc.tensor.matmul(
                U[0][0][:, 0:8], ones_sb[0:1, :], ones_sb[0:1, 0:8],
                start=True, stop=True, skip_group_check=True,
            )

            # ---- main loop: stream protos; scores -> exp -> PE pooling -----
            # proto DRAM is [256, 32, 768]; a tile carries proto pair g for
            # BOTH 128-token blocks: [n, bb, p, e]
            proto_v = proto_d[:].rearrange("(bb n) p e -> n bb p e", n=BLK)
            g0 = 0
            for c, ntiles in enumerate(CHUNK_TILES):
                cp0, cp1 = g0 * PG, (g0 + ntiles) * PG   # proto range of chunk
                t_tiles = []
                for t in range(ntiles):
                    g = g0 + t
                    T2 = ppool.tile([128, NBLK, PG, D_CTX], f16, tag="T")
                    nc.sync.dma_start(
                        out=T2[:], in_=proto_v[:, :, g * PG:(g + 1) * PG, :]
                    )
                    t_tiles.append(T2)
                    for j in range(PG):
                        p = g * PG + j
                        q = p - cp0          # index within this chunk
                        for b in range(NBLK):
                            prod = prodp.tile([128, D_CTX], f16, tag="pr")
                            nc.vector.affine_mul_reduce(
                                out=prod[:],
                                accum_out=scores[c][:, b, q:q + 1],
                                in0=T2[:, b, j, :],
                                in1=qp_sb[:, b, :],
                                scale=1.0,
                                bias=0.0,
                            )

                for b in range(NBLK):
                    s_ch = scores[c][:, b, :]
                    e_ch = expw[c][:, b, :]
                    if c == 0:
                        # fixed exponent frame from chunk 0 (see header)
                        m0 = small.tile([128, 1], f32, tag="m0")
                        nc.vector.tensor_reduce(
                            out=m0[:], in_=s_ch, axis=X, op=Alu.max,
                        )
                        nc.vector.tensor_scalar(
                            negMhat[:, b, :], m0[:], -1.0, -60.0,
                            Alu.mult, Alu.add,
                        )
                        nc.vector.tensor_scalar(
                            clampv[:, b, :], m0[:], 1.0, 140.0,
                            Alu.mult, Alu.add,
                        )
                    else:
                        nc.vector.tensor_scalar(
                            s_ch, s_ch, clampv[:, b, :], None, Alu.min,
                        )
                    nc.scalar.activation(
                        out=e_ch, in_=s_ch, func=Act.Exp,
                        bias=negMhat[:, b, :], scale=1.0,
                        accum_out=zparts[:, b, c:c + 1],
                    )

                # pooling MACs on PE: diag(e_p) @ T_p, PSUM-accumulated
                for t in range(ntiles):
                    T2 = t_tiles[t]
                    for j in range(PG):
                        p = (g0 + t) * PG + j
                        q = p - cp0
                        for b in range(NBLK):
                            e_p = expw[c][:, b, q:q + 1]
                            dg = diagp.tile([128, BLK], bf16, tag="dg")
                            if p in DIAG_ON_ACT:
                                nc.scalar.activation(
                                    out=dg[:], in_=eye_sb[:], func=Act.Copy,
                                    scale=e_p,
                                )
                            else:
                                nc.vector.tensor_scalar(
                                    out=dg[:], in0=eye_sb[:], scalar1=e_p,
                                    scalar2=None, op0=Alu.mult,
                                )
                            for h in range(2):
                                nc.tensor.matmul(
                                    U[b][h][:],
                                    dg[:],
                                    T2[:, b, j, h * EH:(h + 1) * EH],
                                    start=(p == 0),
                                    stop=(p == P - 1),
                                    skip_group_check=True,
                                )
                g0 += ntiles

            # ---- finalize: ctx = U / Z -> f32 out --------------------------
            for b in range(NBLK):
                zsum = small.tile([128, 1], f32, tag="zs")
                nc.vector.tensor_reduce(
                    out=zsum[:], in_=zparts[:, b, :], axis=X, op=Alu.add,
                )
                rinv = small.tile([128, 1], f32, tag="ri")
                nc.vector.reciprocal(out=rinv[:], in_=zsum[:])
                # h=0 on ACT, h=1 on DVE: the two normalizations run in
                # parallel during the latency-critical tail
                nc.scalar.activation(
                    out=outsb[:, b, 0:EH],
                    in_=U[b][0][:], func=Act.Copy, scale=rinv[:],
                )
                nc.sync.dma_start(
                    out=out_d[b * BLK:(b + 1) * BLK, 0:EH],
                    in_=outsb[:, b, 0:EH],
                )
                nc.vector.tensor_scalar(
                    out=outsb[:, b, EH:D_CTX], in0=U[b][1][:],
                    scalar1=rinv[:], scalar2=None, op0=Alu.mult,
                )
                nc.sync.dma_start(
                    out=out_d[b * BLK:(b + 1) * BLK, EH:D_CTX],
                    in_=outsb[:, b, EH:D_CTX],
                )

    nc.compile()
    return nc


def _get_nc():
    global _NC
    if _NC is None:
        _NC = _build()
    return _NC


def _make_in_maps(sent_vecs, proto_vecs, Wq, bq, Wk):
    sent = np.asarray(sent_vecs, dtype=np.float32).reshape(TOK, D_SENT)
    sentT = np.ascontiguousarray(sent.T).astype(np.float16)
    proto = np.asarray(proto_vecs, dtype=np.float32).reshape(TOK, P, D_CTX)
    proto = np.ascontiguousarray(proto.astype(np.float16))
    wq = np.asarray(Wq, dtype=np.float32)
    bq = np.asarray(bq, dtype=np.float32).reshape(1, D_CTX)
    wk = np.asarray(Wk, dtype=np.float32)
    # fold the projection weights host-side: qp = sent @ W + bp
    w = np.ascontiguousarray((wq @ wk.T).astype(np.float16))
    bp = np.ascontiguousarray((bq @ wk.T).astype(np.float16))
    import ml_dtypes
    eye = np.eye(BLK, dtype=np.float32).astype(ml_dtypes.bfloat16)
    in_maps = []
    for i in range(N_CORES):
        sl = slice(i * TPC, (i + 1) * TPC)
        in_maps.append(
            {
                "sentT": np.ascontiguousarray(sentT[:, sl]),
                "proto": np.ascontiguousarray(proto[sl]),
                "w": w,
                "bp": bp,
                "eye": eye,
            }
        )
    return in_maps


def _ensure_ntff_hook():
    """The agent image's antenv lacks axon_hooks; shim it so trace=True
    can capture NTFF profiles via the libaxon ctypes path."""
    try:
        from antenv.axon_hooks import get_axon_ntff_profile_hook  # noqa: F401
        return
    except ImportError:
        pass
    import types

    import antenv
    from trn_agent_boot.trn_boot import _ntff_profile_via_ctypes

    mod = types.ModuleType("antenv.axon_hooks")
    mod._hook = _ntff_profile_via_ctypes("/opt/axon/libaxon_pjrt.so")
    mod.get_axon_ntff_profile_hook = lambda: mod._hook
    mod.set_axon_ntff_profile_hook = lambda h: setattr(mod, "_hook", h)
    sys.modules["antenv.axon_hooks"] = mod
    antenv.axon_hooks = mod


def run(sent_vecs, proto_vecs, Wq, bq, Wk, bk=None, trace=False, **kw):
    """Returns (out[4,512,768] float32, BassKernelResults)."""
    from concourse.bass_utils import run_bass_kernel_spmd

    if trace:
        _ensure_ntff_hook()
    nc = _get_nc()
    in_maps = _make_in_maps(sent_vecs, proto_vecs, Wq, bq, Wk)
    res = run_bass_kernel_spmd(
        nc, in_maps, core_ids=list(range(N_CORES)), trace=trace
    )
    outs = [np.asarray(res.results[i]["out"]) for i in range(N_CORES)]
    full = np.concatenate(outs, axis=0).reshape(B, S, D_CTX).astype(np.float32)
    return full, res


def kernel(sent_vecs, proto_vecs, Wq, bq, Wk, bk=None, **kw):
    out, _ = run(sent_vecs, proto_vecs, Wq, bq, Wk, bk)
    return out


if __name__ == "__main__":
    nc = _get_nc()
    print("build + compile OK")
    from concourse.timeline_sim import TimelineSim
    print("TimelineSim predicted:", TimelineSim(nc).simulate(), "ns")
